# revision 19
# baseline (speedup 1.0000x reference)
"""Trainium2 Bass kernel for a 2-layer GAT (PyG GATConv, concat=True, eval).

Strategy: edge-parallel sharding by dst range across 8 NeuronCores.
 - Host: append self-loops, sort edges by (core, 128-node dst window, src-half),
   pad each window's lo/hi src runs to uniform block counts (SPMD: one program).
 - Device, per core:
   Phase A: replicated node-table build  htab1 = [x @ W1 | x @ W1@Asrc | x @ W1@Adst]
            (+ small per-core dst-side table shard "myadtab").
   Phase B: per dst-window: dma_gather rows by src (lo/hi split keeps idx < 32768),
            gather adst by dst-local; alpha = lrelu(asrc+adst); ex = exp(alpha);
            msg = h_gathered * ex (per-head broadcast); one-hot S from
            tensor_scalar(is_equal(iota + 128*w, dst_rel)); segment softmax-sum as
            PE matmuls accumulating [dst, 256 msg | 8 denom] in PSUM; epilogue
            divides, adds bias, leaky-relus -> x2 (kept in SBUF).
   Phase C: PE-transpose x2, matmul with W2aug -> layer-2 table shard; AllGather.
   Phase D: same edge pipeline on layer-2 table -> output shard.
"""

import sys
import numpy as np

sys.path.insert(0, "/opt/trn_rl_repo")

NEG_SLOPE = 0.2
NCORES = 8
PHASES = "ABCD"   # debug knob: which device phases to build
_cache = {}


def _block_diag_a(a):
    """a: [H, C] -> A: [H*C, H] with A[h*C+c, h] = a[h, c]."""
    H, C = a.shape
    A = np.zeros((H * C, H), np.float32)
    for h in range(H):
        A[h * C:(h + 1) * C, h] = a[h]
    return A


def _pack_edges(src, dst, NPC, NW):
    """Sort/pad edges into the uniform per-core slot structure.

    Returns (BL, BH, per-core dict arrays). Slot layout per core:
    windows of (BL+BH) blocks of 128 slots; first BL blocks = edges with
    src < 32768 ("lo"), then BH blocks = src >= 32768 ("hi"); runs padded
    with (src_idx=0, dst_idx=0, dst_rel=-1).
    """
    E = src.shape[0]
    core = dst // NPC
    win = (dst % NPC) // 128
    hi = (src >= 32768).astype(np.int64)
    key = (core * NW + win) * 2 + hi
    order = np.argsort(key, kind="stable")
    ks, ss, ds = key[order], src[order], dst[order]
    ngroups = NCORES * NW * 2
    cnt = np.bincount(ks, minlength=ngroups)
    cnt2 = cnt.reshape(NCORES, NW, 2)
    BL = int(np.ceil(cnt2[:, :, 0].max() / 128))
    BH = int(max(1, np.ceil(cnt2[:, :, 1].max() / 128)))
    BW = BL + BH
    SL = NW * BW * 128  # slots per core

    # slot base for each group
    gcore, rem = np.divmod(np.arange(ngroups), NW * 2)
    gwin, ghi = np.divmod(rem, 2)
    gbase = gcore * SL + gwin * BW * 128 + ghi * BL * 128
    starts = np.repeat(gbase, cnt)
    # rank within group
    cum = np.cumsum(cnt)
    grp_start_in_order = np.repeat(cum - cnt, cnt)
    rank = np.arange(E) - grp_start_in_order
    slots = starts + rank  # global slot per sorted edge

    src_idx = np.zeros(NCORES * SL, np.int16)
    dst_idx = np.zeros(NCORES * SL, np.int16)
    dst_rel = np.full(NCORES * SL, -1.0, np.float32)
    src_idx[slots] = (ss - 32768 * (ss >= 32768)).astype(np.int16)
    dst_idx[slots] = (ds % NPC).astype(np.int16)
    dst_rel[slots] = (ds % NPC).astype(np.float32)

    percore = []
    for c in range(NCORES):
        s16 = src_idx[c * SL:(c + 1) * SL].reshape(SL // 16, 16).T
        d16 = dst_idx[c * SL:(c + 1) * SL].reshape(SL // 16, 16).T
        percore.append({
            "src_idx": np.ascontiguousarray(np.tile(s16, (8, 1))),
            "dst_idx": np.ascontiguousarray(np.tile(d16, (8, 1))),
            "dst_rel": np.ascontiguousarray(
                dst_rel[c * SL:(c + 1) * SL].reshape(SL // 128, 128).T),
        })
    return BL, BH, percore


def _build_program(meta):
    import concourse.bass as bass
    import concourse.bacc as bacc
    import concourse.tile as tile
    import concourse.mybir as mybir

    f32, i16 = mybir.dt.float32, mybir.dt.int16
    NP, NPC, NW, BL, BH = meta["NP"], meta["NPC"], meta["NW"], meta["BL"], meta["BH"]
    NT = NP // 128            # global node tiles
    BW = BL + BH              # blocks per window
    D1, D2, H = meta["D1"], meta["D2"], meta["H"]
    C1, C2 = D1 // H, D2 // H
    R1, R2 = meta["R1"], meta["R2"]   # table row widths (f32 elems, 64-mult)
    SL = NW * BW * 128
    LO = min(32768, NP)
    HI0 = LO if NP > LO else 0  # hi-gather base (degenerate for small NP)

    nc = bacc.Bacc("TRN2", target_bir_lowering=False, debug=False,
                   num_devices=NCORES)

    def din(name, shape, dt=f32):
        return nc.dram_tensor(name, shape, dt, kind="ExternalInput").ap()

    xTt = din("xTt", [NT, 128, 128])          # x tiles, transposed [fin, node]
    xTmy = din("xTmy", [NW, 128, 128])        # per-core: my node tiles
    w1aug = din("w1aug", [128, R1])
    wad = din("wad", [128, 64])
    w2aug = din("w2aug", [128, 2, D2 + 16])   # K-chunks of [256, D2+16]
    iota_d = din("iota_d", [128, 128])
    ident_d = din("ident_d", [128, 128])
    b1_d = din("b1_d", [128, D1])
    b2_d = din("b2_d", [128, D2])
    sidx_d = din("sidx_d", [128, SL // 16], i16)
    didx_d = din("didx_d", [128, SL // 16], i16)
    drel_d = din("drel_d", [128, SL // 128])
    outy = nc.dram_tensor("outy", [NPC, D2], f32, kind="ExternalOutput").ap()

    with tile.TileContext(nc) as tc:
        with tc.tile_pool(name="dram", bufs=1, space="DRAM") as dram, \
             tc.tile_pool(name="const", bufs=1) as cp, \
             tc.tile_pool(name="persist", bufs=1) as pp, \
             tc.tile_pool(name="build", bufs=3) as bp, \
             tc.tile_pool(name="edge", bufs=2) as ep, \
             tc.tile_pool(name="small", bufs=2) as sp, \
             tc.tile_pool(name="psA", bufs=2, space="PSUM") as psA, \
             tc.tile_pool(name="psB", bufs=2, space="PSUM") as psB, \
             tc.tile_pool(name="psT", bufs=2, space="PSUM") as psT:

            htab1 = dram.tile([NP, R1], f32)
            myadtab = dram.tile([NPC, 64], f32)
            bounce2 = dram.tile([NPC, R2], f32)
            table2 = dram.tile([NP, R2], f32, addr_space="Shared")

            # ---- constants ----
            t_w1 = cp.tile([128, R1], f32)
            t_wad = cp.tile([128, 64], f32)
            nc.sync.dma_start(t_wad[:], wad[:])
            nc.sync.dma_start(t_w1[:], w1aug[:])
            t_w2 = cp.tile([128, 2, D2 + 16], f32)
            nc.sync.dma_start(t_w2[:], w2aug[:])
            t_iota = cp.tile([128, 128], f32)
            nc.sync.dma_start(t_iota[:], iota_d[:])
            t_id = cp.tile([128, 128], f32)
            nc.sync.dma_start(t_id[:], ident_d[:])
            t_b1 = cp.tile([128, D1], f32)
            nc.sync.dma_start(t_b1[:], b1_d[:])
            t_b2 = cp.tile([128, D2], f32)
            nc.sync.dma_start(t_b2[:], b2_d[:])
            t_sidx = pp.tile([128, SL // 16], i16)
            nc.sync.dma_start(t_sidx[:], sidx_d[:])
            t_didx = pp.tile([128, SL // 16], i16)
            nc.sync.dma_start(t_didx[:], didx_d[:])
            t_drel = pp.tile([128, SL // 128], f32)
            nc.sync.dma_start(t_drel[:], drel_d[:])

            x2 = pp.tile([128, NW, D1], f32)          # layer-1 output (my nodes)
            sh2 = pp.tile([128, NW, R2], f32)         # layer-2 table shard
            nc.vector.memset(sh2[:], 0.0)

            PH = meta.get("phases", "ABCD")
            # ---- phase A: replicated htab1 build ----
            for t in range(NT if "A" in PH else 0):
                xt = bp.tile([128, 128], f32, tag="xt")
                nc.sync.dma_start(xt[:], xTt[t])
                ph = psA.tile([128, R1], f32, tag="ph")
                nc.tensor.matmul(ph[:], xt[:], t_w1[:], start=True, stop=True)
                th = bp.tile([128, R1], f32, tag="th")
                nc.scalar.copy(th[:], ph[:])
                nc.sync.dma_start(
                    htab1[:].rearrange("(t p) e -> t p e", p=128)[t], th[:])

            # ---- phase A2: my dst-side table (asrc|adst for my nodes) ----
            for w in range(NW if "A" in PH else 0):
                xt = bp.tile([128, 128], f32, tag="xt")
                nc.sync.dma_start(xt[:], xTmy[w])
                pa = psA.tile([128, 64], f32, tag="ph")
                nc.tensor.matmul(pa[:], xt[:], t_wad[:], start=True, stop=True)
                ta = bp.tile([128, 64], f32, tag="ta")
                nc.scalar.copy(ta[:], pa[:])
                nc.sync.dma_start(
                    myadtab[:].rearrange("(w p) e -> w p e", p=128)[w], ta[:])

            # ---- edge-phase helper ----
            def edge_phase(layer, tab, dtab, D, R, out_epilogue):
                """layer 1: tab=htab1 (row R1), layer 2: tab=table2 (row R2)."""
                asrc_off = D          # asrc at row cols [D, D+8)
                GMAX = 8  # blocks per gather call (<=64 descriptors/engine)

                def gather_calls(out_tile, b0, nblk, in_ap, idx_tile, idx0, elem):
                    o = 0
                    while o < nblk:
                        g = min(GMAX, nblk - o)
                        nc.gpsimd.dma_gather(
                            out_ap=out_tile[:, b0 + o:b0 + o + g, :],
                            in_ap=in_ap,
                            idxs_ap=idx_tile[:, idx0 + o * 8: idx0 + (o + g) * 8],
                            num_idxs=g * 128, num_idxs_reg=g * 128,
                            elem_size=elem)
                        o += g

                for w in range(NW):
                    hg = ep.tile([128, BW, R], f32, tag=f"hg{layer}")
                    gather_calls(hg, 0, BL, tab[0:LO, :], t_sidx, w * (BW * 8), R)
                    gather_calls(hg, BL, BH, tab[HI0:NP, :], t_sidx,
                                 w * (BW * 8) + BL * 8, R)
                    ad = ep.tile([128, BW, 64], f32, tag="ad")
                    gather_calls(ad, 0, BW, dtab, t_didx, w * (BW * 8), 64)
                    # alpha = lrelu(asrc + adst); ex = exp(alpha)
                    al = sp.tile([128, BW, 8], f32, tag="al")
                    nc.vector.tensor_tensor(
                        al[:], hg[:, :, asrc_off:asrc_off + 8],
                        ad[:, :, meta[f"adoff{layer}"]:meta[f"adoff{layer}"] + 8],
                        mybir.AluOpType.add)
                    al2 = sp.tile([128, BW, 8], f32, tag="al2")
                    nc.vector.tensor_scalar_mul(al2[:], al[:], NEG_SLOPE)
                    nc.vector.tensor_tensor(al2[:], al2[:], al[:],
                                            mybir.AluOpType.max)
                    ex = sp.tile([128, BW, 8], f32, tag="ex")
                    nc.scalar.activation(ex[:], al2[:],
                                         mybir.ActivationFunctionType.Exp)
                    # msg = hg * ex (per-head bcast), in place; ex -> denom cols
                    CC = D // H
                    nc.vector.tensor_tensor(
                        hg[:, :, 0:D].rearrange("p b (h c) -> p b h c", h=H),
                        hg[:, :, 0:D].rearrange("p b (h c) -> p b h c", h=H),
                        ex[:].rearrange("p b (h o) -> p b h o", o=1)
                             .broadcast_to([128, BW, H, CC]),
                        mybir.AluOpType.mult)
                    nc.scalar.copy(hg[:, :, D:D + 8], ex[:])
                    # one-hot segment matmuls
                    ps = psB.tile([128, D + 8], f32, tag="ps")
                    for b in range(BW):
                        S = ep.tile([128, 128], f32, tag="S", bufs=4)
                        nc.vector.tensor_scalar(
                            S[:], t_iota[:], float(128 * w),
                            t_drel[:, w * BW + b: w * BW + b + 1],
                            mybir.AluOpType.add,
                            mybir.AluOpType.is_equal)
                        nc.tensor.matmul(ps[:], S[:], hg[:, b, 0:D + 8],
                                         start=(b == 0), stop=(b == BW - 1))
                    out_epilogue(w, ps)

            # ---- phase B epilogue: x2 = lrelu(msg/denom + b1) ----
            def epi1(w, ps):
                rc = sp.tile([128, 8], f32, tag="rc")
                nc.vector.reciprocal(rc[:], ps[:, D1:D1 + 8])
                y = sp.tile([128, D1], f32, tag="y")
                nc.vector.tensor_tensor(
                    y[:].rearrange("p (h c) -> p h c", h=H),
                    ps[:, 0:D1].rearrange("p (h c) -> p h c", h=H),
                    rc[:].rearrange("p (h o) -> p h o", o=1)
                         .broadcast_to([128, H, C1]),
                    mybir.AluOpType.mult)
                nc.vector.tensor_tensor(y[:], y[:], t_b1[:], mybir.AluOpType.add)
                y2 = sp.tile([128, D1], f32, tag="y2")
                nc.vector.tensor_scalar_mul(y2[:], y[:], NEG_SLOPE)
                nc.vector.tensor_tensor(x2[:, w, :], y2[:], y[:],
                                        mybir.AluOpType.max)

            if "B" in PH:
                edge_phase(1, htab1, myadtab[:], D1, R1, epi1)
            else:
                nc.vector.memset(x2[:], 0.0)

            # ---- phase C: layer-2 table shard + AllGather ----
            for w in range(NW if "C" in PH else 0):
                pt = psT.tile([128, 128], f32, tag="pt")
                nc.tensor.transpose(pt[:], x2[:, w, 0:128], t_id[:])
                xta = bp.tile([128, 128], f32, tag="xta")
                nc.scalar.copy(xta[:], pt[:])
                pt2 = psT.tile([128, 128], f32, tag="pt")
                nc.tensor.transpose(pt2[:], x2[:, w, 128:256], t_id[:])
                xtb = bp.tile([128, 128], f32, tag="xtb")
                nc.scalar.copy(xtb[:], pt2[:])
                p2 = psA.tile([128, D2 + 16], f32, tag="ph")
                nc.tensor.matmul(p2[:], xta[:], t_w2[:, 0, :], start=True,
                                 stop=False)
                nc.tensor.matmul(p2[:], xtb[:], t_w2[:, 1, :], start=False,
                                 stop=True)
                nc.scalar.copy(sh2[:, w, 0:D2 + 16], p2[:])
            nc.sync.dma_start(
                bounce2[:].rearrange("(w p) e -> p w e", p=128), sh2[:])
            if "C" in PH:
                nc.gpsimd.collective_compute(
                    "AllGather", mybir.AluOpType.bypass,
                    replica_groups=[list(range(NCORES))],
                    ins=[bounce2.opt()], outs=[table2.opt()])

            # ---- phase D epilogue: out = msg/denom + b2 ----
            def epi2(w, ps):
                rc = sp.tile([128, 8], f32, tag="rc2")
                nc.vector.reciprocal(rc[:], ps[:, D2:D2 + 8])
                y = sp.tile([128, D2], f32, tag="yo")
                nc.vector.tensor_tensor(
                    y[:].rearrange("p (h c) -> p h c", h=H),
                    ps[:, 0:D2].rearrange("p (h c) -> p h c", h=H),
                    rc[:].rearrange("p (h o) -> p h o", o=1)
                         .broadcast_to([128, H, C2]),
                    mybir.AluOpType.mult)
                nc.vector.tensor_tensor(y[:], y[:], t_b2[:], mybir.AluOpType.add)
                nc.sync.dma_start(
                    outy.rearrange("(w p) e -> w p e", p=128)[w], y[:])

            if "D" in PH:
                edge_phase(2, table2, bounce2[:], D2, R2, epi2)
            else:
                yz = sp.tile([128, D2], f32, tag="yz")
                nc.vector.memset(yz[:], 0.0)
                for w in range(NW):
                    nc.sync.dma_start(
                        outy.rearrange("(w p) e -> w p e", p=128)[w], yz[:])

    nc.compile()
    return nc


def kernel(x, edge_index, W1, a_src1, a_dst1, b1, W2, a_src2, a_dst2, b2):
    from concourse.bass_utils import run_bass_kernel_spmd

    x = np.asarray(x, np.float32)
    edge_index = np.asarray(edge_index)
    N, FIN = x.shape
    assert FIN == 128
    H, C1 = np.asarray(a_src1).shape
    D1 = np.asarray(W1).shape[1]
    D2 = np.asarray(W2).shape[1]
    C2 = D2 // H

    NPC = int(np.ceil(N / NCORES / 128)) * 128
    NW = NPC // 128
    NP = NPC * NCORES
    R1 = int(np.ceil((D1 + 16) / 64)) * 64
    R2 = int(np.ceil((D2 + 16) / 64)) * 64

    loops = np.arange(N, dtype=np.int64)
    src = np.concatenate([edge_index[0].astype(np.int64), loops])
    dst = np.concatenate([edge_index[1].astype(np.int64), loops])
    BL, BH, percore = _pack_edges(src, dst, NPC, NW)

    meta = {"NP": NP, "NPC": NPC, "NW": NW, "BL": BL, "BH": BH,
            "D1": D1, "D2": D2, "H": H, "R1": R1, "R2": R2,
            "adoff1": 8, "adoff2": D2 + 8, "phases": PHASES}
    key = tuple(sorted(meta.items()))
    if key not in _cache:
        _cache[key] = _build_program(meta)
    nc = _cache[key]

    # host-side constant prep
    A1s, A1d = _block_diag_a(np.asarray(a_src1, np.float32)), \
        _block_diag_a(np.asarray(a_dst1, np.float32))
    A2s, A2d = _block_diag_a(np.asarray(a_src2, np.float32)), \
        _block_diag_a(np.asarray(a_dst2, np.float32))
    W1 = np.asarray(W1, np.float32)
    W2 = np.asarray(W2, np.float32)
    w1aug = np.concatenate(
        [W1, W1 @ A1s, W1 @ A1d,
         np.zeros((FIN, R1 - D1 - 16), np.float32)], 1)          # [128, R1]
    wad = np.concatenate(
        [W1 @ A1s, W1 @ A1d, np.zeros((FIN, 48), np.float32)], 1)  # [128, 64]
    w2aug_full = np.concatenate([W2, W2 @ A2s, W2 @ A2d], 1)     # [256, D2+16]
    w2aug = np.ascontiguousarray(
        w2aug_full.reshape(2, 128, D2 + 16).transpose(1, 0, 2))  # [128,2,D2+16]

    x_pad = np.zeros((NP, FIN), np.float32)
    x_pad[:N] = x
    xTt = np.ascontiguousarray(
        x_pad.reshape(NP // 128, 128, FIN).transpose(0, 2, 1))   # [NT,fin,node]
    iota_np = np.ascontiguousarray(
        np.tile(np.arange(128, dtype=np.float32), (128, 1)))
    ident_np = np.eye(128, dtype=np.float32)
    b1rep = np.ascontiguousarray(
        np.tile(np.asarray(b1, np.float32)[None, :], (128, 1)))
    b2rep = np.ascontiguousarray(
        np.tile(np.asarray(b2, np.float32)[None, :], (128, 1)))

    in_maps = []
    for c in range(NCORES):
        pc = percore[c]
        in_maps.append({
            "xTt": xTt,
            "xTmy": np.ascontiguousarray(xTt[c * NW:(c + 1) * NW]),
            "w1aug": w1aug, "wad": wad, "w2aug": w2aug,
            "iota_d": iota_np, "ident_d": ident_np,
            "b1_d": b1rep, "b2_d": b2rep,
            "sidx_d": pc["src_idx"], "didx_d": pc["dst_idx"],
            "drel_d": pc["dst_rel"],
        })
    global _last_in_maps
    _last_in_maps = in_maps
    import os
    if os.environ.get("KERNEL_SIM"):
        from concourse.bass_interp import MultiCoreSim
        sim = MultiCoreSim(nc, num_cores=NCORES, trace=False,
                           require_finite=False, require_nnan=False)
        for c, core in sim.cores.items():
            for k, v in in_maps[c].items():
                core.tensor(k)[:] = v
        sim.simulate(check_with_hw=False)
        out = np.concatenate(
            [sim.cores[c].tensor("outy").copy() for c in range(NCORES)], 0)
        return np.ascontiguousarray(out[:N])
    res = run_bass_kernel_spmd(nc, in_maps, core_ids=list(range(NCORES)))
    out = np.concatenate([res.results[c]["outy"] for c in range(NCORES)], 0)
    return np.ascontiguousarray(out[:N])
